# revision 30
# baseline (speedup 1.0000x reference)
"""Multi-head attention (B=4, S=2048, D=1024, H=16) on 8 Trainium2 cores.

Sharding: data parallel on batch (4) x tensor parallel on heads (2 halves of
8 heads). Core c handles batch c//2 and head-half c%2: column-parallel
w_q/w_k/w_v (512 out dims), local attention over its 8 heads, row-parallel
w_o (its 512 hd columns) producing a full [2048, 1024] partial that the host
sums across the two halves (plus b_o).

On-device layout is feature-on-partitions throughout ("transposed"):
  qP/kP: [dout 512 -> 4 ptiles, seq 2048] bf16   (projection form B)
  scores S.T: [keys, queries] via paired K=64 matmuls (head pair at PE row
  offsets 0/64 with tile_position) into a 2-bank PSUM tile, one wide exp ACT
  AV: O.T accumulation with V_aug ones-column producing row sums; normalize
  via DVE fast reciprocal + GpSimd partition-broadcast.

Restructure vs the original baseline (398us -> ~388us):
  - input DMAs reordered (wk, kT pair-0, wq, q-chunks first; V and the
    rest interleaved behind) + minimal ramp so the first EXP fires ~23us
    in instead of ~45us
  - one flat 256-step pipeline at the EXP cadence; no per-block filler
    drains; AV jobs ride a tapering lag (8 steps early while the V
    projection streams in JIT, 5 in steady state, 2 at the end) so a
    v-starved AV can never head-of-line-block the scores/EXP stream
  - V input arrives as host-pretransposed per-seq-tile chunks [st, p, o*f]
    so each v chunk is one contiguous 2KB/partition DMA
  - lean tail: atc snapshot for dt0-2 of qb3, their out-proj partials
    precomputed (bf16) during the last steps, only 8 dt3 matmuls + adds
    gated on the final normalize

Dead ends measured on this hardware (do not retry): same-PSUM concurrent
row-split accumulation and 64-wide col-tiling both hang the PE
(NRT_EXEC_UNIT_UNRECOVERABLE; col-group 3 HW bug); matmul outputs cannot
span a PSUM bank (so 1024-wide moving with f32 psum is illegal); fp8
anywhere on the data path blows the 2e-2 error budget (~5% operand noise
passes straight through softmax); DVE has no exp/pow in hardware, so the
33.5M exps/core stay on ScalarE (~272us busy = the cadence floor); the
PSUM 8-bank budget (4 scores + 2 AV + 2 proj) pins the 512-query x
2-head step tiling and the [128,1024] EXP size.
"""

import time
from collections import defaultdict
from contextlib import ExitStack

import ml_dtypes
import numpy as np

import concourse.bass as bass
import concourse.mybir as mybir
import concourse.tile as tile
from concourse import bacc
from concourse.bass import ds, ts
from concourse.bass_utils import run_bass_kernel_spmd

F32 = mybir.dt.float32
BF16 = mybir.dt.bfloat16
EXP = mybir.ActivationFunctionType.Exp
MULT = mybir.AluOpType.mult
ADD = mybir.AluOpType.add
BF = ml_dtypes.bfloat16

B, S, D, H, DH = 4, 2048, 1024, 16, 64
HALF = D // 2          # 512 douts per core
DT = HALF // 128       # 4 dout tiles
DIN = D // 128         # 8 din tiles
QB = S // 512          # 4 query blocks
KT = S // 128          # 16 key tiles / seq tiles
STEPS = QB * DT * KT   # 256

TRACE = False
LAST_EXEC_NS = None
LAST_TRACE = None
_NC = None


def _build():
    nc = bacc.Bacc("TRN2", target_bir_lowering=False, debug=False,
                   num_devices=8, name="mha")

    qT_d = nc.dram_tensor("qT", [D, S], BF16, kind="ExternalInput")
    kT_d = nc.dram_tensor("kT", [D, S], BF16, kind="ExternalInput")
    vR_d = nc.dram_tensor("vR", [KT, 128, D], BF16, kind="ExternalInput")
    wq_d = nc.dram_tensor("wq", [D, HALF], BF16, kind="ExternalInput")
    wk_d = nc.dram_tensor("wk", [D, HALF], BF16, kind="ExternalInput")
    wv_d = nc.dram_tensor("wv", [D, HALF], BF16, kind="ExternalInput")
    wo_d = nc.dram_tensor("wo", [HALF, D], BF16, kind="ExternalInput")
    bq_d = nc.dram_tensor("bq", [1, HALF], BF16, kind="ExternalInput")
    bk_d = nc.dram_tensor("bk", [1, HALF], BF16, kind="ExternalInput")
    bv_d = nc.dram_tensor("bv", [1, HALF], BF16, kind="ExternalInput")
    out_d = nc.dram_tensor("out", [S, D], BF16, kind="ExternalOutput")

    stk = ExitStack()
    with tile.TileContext(nc) as tc:
        persist = stk.enter_context(tc.tile_pool(name="persist", bufs=1))
        kch = stk.enter_context(tc.tile_pool(name="kch", bufs=16))
        qch = stk.enter_context(tc.tile_pool(name="qch", bufs=16))
        vch = stk.enter_context(tc.tile_pool(name="vch", bufs=5))
        pTp = stk.enter_context(tc.tile_pool(name="pTp", bufs=11))
        otsb = stk.enter_context(tc.tile_pool(name="otsb", bufs=2))
        nrm = stk.enter_context(tc.tile_pool(name="nrm", bufs=1))
        outsb = stk.enter_context(tc.tile_pool(name="outsb", bufs=2))
        tailp = stk.enter_context(tc.tile_pool(name="tailp", bufs=8))
        ps_pair = stk.enter_context(tc.tile_pool(name="ps_pair", bufs=2, space="PSUM"))
        ps_ot = stk.enter_context(tc.tile_pool(name="ps_ot", bufs=2, space="PSUM"))
        ps_proj = stk.enter_context(tc.tile_pool(name="ps_proj", bufs=2, space="PSUM"))

        # --- persistent SBUF ---
        wq_sb = persist.tile([128, DIN, HALF], BF16)
        wk_sb = persist.tile([128, DIN, HALF], BF16)
        wv_sb = persist.tile([128, DIN, HALF], BF16)
        wo_sb = persist.tile([128, DT, D], BF16)
        bv_sb = persist.tile([1, HALF], BF16)
        bqP_bf = persist.tile([128, DT], BF16)
        bkP_bf = persist.tile([128, DT], BF16)
        bqP = persist.tile([128, DT], F32)
        bkP = persist.tile([128, DT], F32)
        bvb = persist.tile([128, HALF], BF16)
        qP = persist.tile([128, DT, S], BF16)
        kP = persist.tile([128, DT, S], BF16)
        v_aug = persist.tile([128, KT, 8 * 65], BF16)
        attnT = persist.tile([128, DT, S], BF16)
        atc = persist.tile([128, 3, 512], BF16)

        # ---- head DMAs, ordered so the first EXP fires ASAP ----
        nc.sync.dma_start(bkP_bf[:], bk_d[:].rearrange("a (o p) -> p (a o)", p=128))
        nc.sync.dma_start(bqP_bf[:], bq_d[:].rearrange("a (o p) -> p (a o)", p=128))
        nc.sync.dma_start(bv_sb[:], bv_d[:])
        nc.vector.tensor_copy(bkP[:], bkP_bf[:])
        nc.vector.tensor_copy(bqP[:], bqP_bf[:])
        nc.gpsimd.partition_broadcast(bvb[:], bv_sb[0:1, :])
        nc.sync.dma_start(wk_sb[:], wk_d[:].rearrange("(o p) n -> p o n", p=128))
        kchunks = {}
        for d in range(DIN):  # kb-pair 0: keys 0..1023
            t = kch.tile([128, 1024], BF16, tag="kch")
            nc.sync.dma_start(
                t[:], kT_d[:].rearrange("(o p) f -> o p f", p=128)[d][:, 0:1024])
            kchunks[(d, 0)] = t
        nc.sync.dma_start(wq_sb[:], wq_d[:].rearrange("(o p) n -> p o n", p=128))
        qchunks = {}

        def load_qchunks(qb):
            for d in range(DIN):
                t = qch.tile([128, 512], BF16, tag="qch")
                nc.sync.dma_start(
                    t[:], qT_d[:].rearrange("(o p) f -> o p f", p=128)[d][:, ts(qb, 512)])
                qchunks[(d, qb)] = t

        load_qchunks(0)
        nc.sync.dma_start(wv_sb[:], wv_d[:].rearrange("(o p) n -> p o n", p=128))
        nc.vector.memset(v_aug[:], 1.0)
        vchunks = {}

        def load_vchunk(st):
            t = vch.tile([128, DIN, 128], BF16, tag="vch")
            nc.sync.dma_start(
                t[:], vR_d[st].rearrange("p (o f) -> p o f", o=DIN))
            vchunks[st] = t

        def load_kpair1(ds_, de):
            for d in range(ds_, de):  # kb-pair 1: keys 1024..2047
                t = kch.tile([128, 1024], BF16, tag="kch")
                nc.sync.dma_start(
                    t[:], kT_d[:].rearrange("(o p) f -> o p f", p=128)[d][:, 1024:2048])
                kchunks[(d, 1)] = t

        for st in range(3):
            load_vchunk(st)
        load_kpair1(0, 4)
        for st in range(3, 5):
            load_vchunk(st)
        load_kpair1(4, 8)
        load_qchunks(1)

        # ---- projection emitters (item lists for the drip-feed) ----
        def kproj_items(dt, kb):
            """K projection, one dout tile x 512 keys + bias writeback."""
            state = {}

            def mk(d0):
                def mm():
                    if d0 == 0:
                        state["ps"] = ps_proj.tile([128, 512], F32, tag="proj", name="proj_ps")
                    for d in (d0, d0 + 1):
                        nc.tensor.matmul(state["ps"][:], wk_sb[:, d, ts(dt, 128)],
                                         kchunks[(d, kb // 2)][:, ts(kb % 2, 512)],
                                         start=(d == 0), stop=(d == DIN - 1))
                return mm

            def wb():
                nc.vector.tensor_scalar(kP[:, dt, ts(kb, 512)], state["ps"][:],
                                        bkP[:, dt:dt + 1], None, op0=ADD)

            return [mk(0), mk(2), mk(4), mk(6), wb]

        def qproj_items(dt, qb):
            """Q projection, one dout tile x 512 queries + bias writeback."""
            state = {}

            def mk(d0):
                def mm():
                    if d0 == 0:
                        state["ps"] = ps_proj.tile([128, 512], F32, tag="proj", name="proj_ps")
                    for d in (d0, d0 + 1):
                        nc.tensor.matmul(state["ps"][:], wq_sb[:, d, ts(dt, 128)],
                                         qchunks[(d, qb)][:],
                                         start=(d == 0), stop=(d == DIN - 1))
                return mm

            def wb():
                nc.vector.tensor_scalar(qP[:, dt, ts(qb, 512)], state["ps"][:],
                                        bqP[:, dt:dt + 1], None, op0=ADD)

            return [mk(0), mk(2), mk(4), mk(6), wb]

        def vproj_items(st):
            """V projection for seq tile st (all 8 local heads): 8 matmuls
            + biased writeback into v_aug's per-head 64-col slots."""
            state = {}

            def mk(d0):
                def mm():
                    if d0 == 0:
                        state["ps"] = ps_proj.tile([128, 512], F32, tag="proj", name="proj_ps")
                    for d in (d0, d0 + 1):
                        nc.tensor.matmul(state["ps"][:], vchunks[st][:, d, :],
                                         wv_sb[:, d, :],
                                         start=(d == 0), stop=(d == DIN - 1))
                return mm

            def wb():
                nc.vector.tensor_tensor(
                    v_aug[:, st].rearrange("p (h c) -> p h c", h=8)[:, :, 0:64],
                    state["ps"][:].rearrange("p (h c) -> p h c", h=8),
                    bvb[:].rearrange("p (h c) -> p h c", h=8), ADD)

            return [mk(0), mk(2), mk(4), mk(6), wb]

        def outproj_items(qb):
            """Out-projection for query block qb: per (seq tile, dout half),
            4 matmuls + writeback/DMA."""
            items = []
            for j in range(4):
                st = qb * 4 + j
                for half in range(2):
                    state = {}

                    def mk_a(st=st, half=half, state=state):
                        def mm():
                            state["ps"] = ps_proj.tile([128, 512], F32, tag="proj", name="proj_ps")
                            for dt in (0, 1):
                                nc.tensor.matmul(state["ps"][:],
                                                 attnT[:, dt, ts(st, 128)],
                                                 wo_sb[:, dt, ts(half, 512)],
                                                 start=(dt == 0), stop=False)
                        return mm

                    def mk_b(st=st, half=half, state=state):
                        def mm():
                            for dt in (2, 3):
                                nc.tensor.matmul(state["ps"][:],
                                                 attnT[:, dt, ts(st, 128)],
                                                 wo_sb[:, dt, ts(half, 512)],
                                                 start=False, stop=(dt == 3))
                        return mm

                    def wb(st=st, half=half, state=state):
                        osb = outsb.tile([128, 512], BF16, tag="osb")
                        nc.vector.tensor_copy(osb[:], state["ps"][:])
                        nc.sync.dma_start(
                            out_d[ds(st * 128, 128), ts(half, 512)], osb[:])

                    items += [mk_a(), mk_b(), wb]
            return items

        # ---- ramp: the minimum needed for scores step 0 ----
        for it in kproj_items(0, 0) + kproj_items(0, 1):
            it()
        for it in qproj_items(0, 0):
            it()

        # ---- filler schedule: step -> list of closures ----
        sched = defaultdict(list)

        def add(s0, items, per_step):
            s = s0
            n = 0
            for it in items:
                sched[s].append(it)
                n += 1
                if n >= per_step:
                    s += 1
                    n = 0

        # kP: dt needed from block hp=dt (step 16*dt); kbp1 of dt0 by step 8
        add(4, kproj_items(0, 2) + kproj_items(0, 3), 2)
        add(9, kproj_items(1, 0) + kproj_items(1, 1)
            + kproj_items(1, 2) + kproj_items(1, 3), 3)
        add(16, kproj_items(2, 0) + kproj_items(2, 1)
            + kproj_items(2, 2) + kproj_items(2, 3), 2)
        add(32, kproj_items(3, 0) + kproj_items(3, 1)
            + kproj_items(3, 2) + kproj_items(3, 3), 2)
        # V: group st feeds AV job (hp0, kt=st) which rides ~step st+6
        for st in range(16):
            add(st, vproj_items(st), 5)
        sched[2].append(lambda: [load_vchunk(st) for st in range(5, 9)])
        sched[6].append(lambda: [load_vchunk(st) for st in range(9, 13)])
        sched[10].append(lambda: [load_vchunk(st) for st in range(13, 16)])
        # Q: (dt, qb0) needed by block hp=dt; qb1.. later
        add(10, qproj_items(1, 0), 2)
        add(24, qproj_items(2, 0), 2)
        add(40, qproj_items(3, 0), 2)
        sched[20].append(lambda: nc.sync.dma_start(
            wo_sb[:], wo_d[:].rearrange("(o p) n -> p o n", p=128)))
        add(44, qproj_items(0, 1), 2)
        add(52, qproj_items(1, 1), 2)
        add(72, qproj_items(2, 1), 2)
        add(88, qproj_items(3, 1), 2)
        sched[64].append(lambda: load_qchunks(2))
        sched[140].append(lambda: load_qchunks(3))
        add(100, qproj_items(0, 2), 2)
        add(108, qproj_items(1, 2), 2)
        add(116, qproj_items(2, 2), 2)
        add(124, qproj_items(3, 2), 2)
        add(156, qproj_items(0, 3), 2)
        add(164, qproj_items(1, 3), 2)
        add(172, qproj_items(2, 3), 2)
        add(180, qproj_items(3, 3), 2)
        # out-projection of completed query blocks
        add(66, outproj_items(0), 1)
        add(130, outproj_items(1), 1)
        add(194, outproj_items(2), 1)

        # ---- AV job machinery (tapering lag behind the scores stream) ----
        pp_store = {}
        block_ot = {}
        av_state = {"idx": 0}

        def emit_normalize(b):
            qb_, hp_ = divmod(b, DT)
            otA, otB = block_ot.pop(b)
            sm = nrm.tile([1, 1024], F32, tag="sums")
            nc.vector.tensor_copy(sm[0:1, 0:512], otA[64:65, :])
            nc.vector.tensor_copy(sm[0:1, 512:1024], otB[64:65, :])
            oa = otsb.tile([128, 512], F32, tag="ot_sb")
            ob = otsb.tile([128, 512], F32, tag="ot_sb")
            nc.vector.tensor_copy(oa[0:64, :], otA[0:64, :])
            nc.vector.tensor_copy(ob[0:64, :], otB[0:64, :])
            r = nrm.tile([1, 1024], F32, tag="recip")
            nc.vector.reciprocal_approx_fast(r[0:1, :], sm[0:1, :])
            rb = nrm.tile([64, 1024], F32, tag="rb")
            nc.gpsimd.partition_broadcast(rb[:], r[0:1, :])
            nc.vector.tensor_tensor(attnT[0:64, hp_, ts(qb_, 512)],
                                    oa[0:64, :], rb[:, 0:512], MULT)
            nc.vector.tensor_tensor(attnT[64:128, hp_, ts(qb_, 512)],
                                    ob[0:64, :], rb[:, 512:1024], MULT)

        def emit_av():
            b, kt = divmod(av_state["idx"], KT)
            qb_, hp_ = divmod(b, DT)
            if kt == 0:
                block_ot[b] = (ps_ot.tile([128, 512], F32, tag="ot", name="otA"),
                               ps_ot.tile([128, 512], F32, tag="ot", name="otB"))
            otA, otB = block_ot[b]
            p = pp_store.pop((b, kt))
            nc.tensor.matmul(otA[0:65, :], v_aug[:, kt, ds(2 * hp_ * 65, 65)],
                             p[:, 0:512], start=(kt == 0), stop=(kt == KT - 1))
            nc.tensor.matmul(otB[0:65, :], v_aug[:, kt, ds((2 * hp_ + 1) * 65, 65)],
                             p[:, 512:1024], start=(kt == 0), stop=(kt == KT - 1))
            av_state["idx"] += 1
            if kt == KT - 1 and b < QB * DT - 1:
                emit_normalize(b)

        def av_target(s):
            if s < 30:
                lag = 8
            elif s < 40:
                lag = 6
            elif s < 248:
                lag = 5
            else:
                lag = 2
            return s - lag

        # ---- tail partials: dt0-2 of qb3's out-proj, dripped pre-tail ----
        tail_parts = {}

        def tail_partial_item(j):
            def f():
                psa = ps_proj.tile([128, 512], F32, tag="proj", name="tail_pa")
                psb = ps_proj.tile([128, 512], F32, tag="proj", name="tail_pb")
                for dt in (0, 1, 2):
                    nc.tensor.matmul(psa[:], atc[:, dt, ds(j * 128, 128)],
                                     wo_sb[:, dt, 0:512],
                                     start=(dt == 0), stop=(dt == 2))
                    nc.tensor.matmul(psb[:], atc[:, dt, ds(j * 128, 128)],
                                     wo_sb[:, dt, 512:1024],
                                     start=(dt == 0), stop=(dt == 2))
                tpa = tailp.tile([128, 512], BF16, tag="tp", name="tpa")
                tpb = tailp.tile([128, 512], BF16, tag="tp", name="tpb")
                nc.vector.tensor_copy(tpa[:], psa[:])
                nc.vector.tensor_copy(tpb[:], psb[:])
                tail_parts[(j, 0)] = tpa
                tail_parts[(j, 1)] = tpb
            return f

        sched[243].append(
            lambda: nc.vector.tensor_copy(atc[:], attnT[:, 0:3, ts(QB - 1, 512)]))
        for j in range(4):
            sched[245 + 2 * j].append(tail_partial_item(j))

        # ---- the flat 256-step pipeline ----
        for s in range(STEPS):
            qb, r = divmod(s, DT * KT)
            hp, kt = divmod(r, KT)
            pair = ps_pair.tile([128, 1024], F32, tag="pair")
            nc.tensor.matmul(pair[:, 0:512],
                             kP[0:64, hp, ts(kt, 128)],
                             qP[0:64, hp, ts(qb, 512)],
                             start=True, stop=True, tile_position=(0, 0))
            nc.tensor.matmul(pair[:, 512:1024],
                             kP[64:128, hp, ts(kt, 128)],
                             qP[64:128, hp, ts(qb, 512)],
                             start=True, stop=True, tile_position=(64, 0))
            p = pTp.tile([128, 1024], BF16, tag="pT")
            nc.scalar.activation(p[:], pair[:], EXP, scale=0.125)
            pp_store[(qb * DT + hp, kt)] = p
            for f in sched.pop(s, []):
                f()
            while av_state["idx"] <= av_target(s):
                emit_av()

        # ---- tail: last AV jobs, striped final normalize, dt3-only
        # out-proj matmuls added onto the precomputed partials ----
        while av_state["idx"] < QB * DT * KT:
            emit_av()
        b_last = QB * DT - 1
        otA, otB = block_ot.pop(b_last)
        sm = nrm.tile([1, 1024], F32, tag="sums")
        nc.vector.tensor_copy(sm[0:1, 0:512], otA[64:65, :])
        nc.vector.tensor_copy(sm[0:1, 512:1024], otB[64:65, :])
        r = nrm.tile([1, 1024], F32, tag="recip")
        nc.vector.reciprocal_approx_fast(r[0:1, :], sm[0:1, :])
        rb = nrm.tile([64, 1024], F32, tag="rb")
        nc.gpsimd.partition_broadcast(rb[:], r[0:1, :])
        oa = otsb.tile([128, 512], F32, tag="ot_sb")
        ob = otsb.tile([128, 512], F32, tag="ot_sb")
        nc.vector.tensor_copy(oa[0:64, :], otA[0:64, :])
        nc.vector.tensor_copy(ob[0:64, :], otB[0:64, :])
        for j in range(4):
            st = (QB - 1) * 4 + j
            nc.vector.tensor_tensor(
                attnT[0:64, 3, ds((QB - 1) * 512 + j * 128, 128)],
                oa[0:64, ts(j, 128)], rb[:, ts(j, 128)], MULT)
            nc.vector.tensor_tensor(
                attnT[64:128, 3, ds((QB - 1) * 512 + j * 128, 128)],
                ob[0:64, ts(j, 128)], rb[:, ds(512 + j * 128, 128)], MULT)
            for half in range(2):
                tps = ps_proj.tile([128, 512], F32, tag="proj", name="tail_ps")
                nc.tensor.matmul(tps[:], attnT[:, 3, ts(st, 128)],
                                 wo_sb[:, 3, ts(half, 512)],
                                 start=True, stop=True)
                osb = outsb.tile([128, 512], BF16, tag="osb")
                nc.vector.tensor_tensor(osb[:], tail_parts[(j, half)][:],
                                        tps[:], ADD)
                nc.sync.dma_start(out_d[ds(st * 128, 128), ts(half, 512)], osb[:])

        stk.close()

    nc.finalize()
    return nc


def kernel(q, k, v, mask, w_q, b_q, w_k, b_k, w_v, b_v, w_o, b_o):
    global _NC, LAST_EXEC_NS, LAST_TRACE
    if _NC is None:
        _NC = _build()
    nc = _NC

    q = np.asarray(q, np.float32)
    k = np.asarray(k, np.float32)
    v = np.asarray(v, np.float32)
    w_q = np.asarray(w_q, np.float32)
    w_k = np.asarray(w_k, np.float32)
    w_v = np.asarray(w_v, np.float32)
    w_o = np.asarray(w_o, np.float32)
    b_q = np.asarray(b_q, np.float32)
    b_k = np.asarray(b_k, np.float32)
    b_v = np.asarray(b_v, np.float32)
    b_o = np.asarray(b_o, np.float32)

    in_maps = []
    for c in range(8):
        b, hf = divmod(c, 2)
        sl = slice(hf * HALF, (hf + 1) * HALF)
        # vR[st, p, o, f] = v[b][st*128+f, o*128+p]: one contiguous
        # 2KB/partition DMA per seq tile
        vR = np.ascontiguousarray(
            v[b].reshape(KT, 128, DIN, 128).transpose(0, 3, 2, 1)).astype(BF)
        in_maps.append({
            "qT": q[b].T.astype(BF),
            "kT": k[b].T.astype(BF),
            "vR": vR,
            "wq": w_q[sl, :].T.astype(BF),
            "wk": w_k[sl, :].T.astype(BF),
            "wv": w_v[sl, :].T.astype(BF),
            "wo": w_o[:, sl].T.astype(BF),
            "bq": b_q[sl].reshape(1, HALF).astype(BF),
            "bk": b_k[sl].reshape(1, HALF).astype(BF),
            "bv": b_v[sl].reshape(1, HALF).astype(BF),
        })

    kwargs = {}
    if TRACE:
        kwargs = dict(trace=True, trace_cores=[0])
    try:
        res = run_bass_kernel_spmd(nc, in_maps, core_ids=list(range(8)), **kwargs)
    except Exception:
        # transient device wedge usually clears on retry
        time.sleep(2.0)
        res = run_bass_kernel_spmd(nc, in_maps, core_ids=list(range(8)), **kwargs)
    if TRACE:
        LAST_EXEC_NS = res.exec_time_ns
        LAST_TRACE = res.instructions_and_trace[1] if res.instructions_and_trace else None

    out = np.empty((B, S, D), np.float32)
    for b in range(B):
        out[b] = (res.results[2 * b]["out"].astype(np.float32)
                  + res.results[2 * b + 1]["out"].astype(np.float32)
                  + b_o[None, :])
    return out


# revision 31
# speedup vs baseline: 1.0185x; 1.0185x over previous
"""Multi-head attention (B=4, S=2048, D=1024, H=16) on 8 Trainium2 cores.

Sharding: data parallel on batch (4) x tensor parallel on heads (2 halves of
8 heads). Core c handles batch c//2 and head-half c%2: column-parallel
w_q/w_k/w_v (512 out dims), local attention over its 8 heads, row-parallel
w_o (its 512 hd columns) producing a full [2048, 1024] partial that the host
sums across the two halves (plus b_o).

On-device layout is feature-on-partitions throughout ("transposed"):
  qP/kP: [dout 512 -> 4 ptiles, seq 2048] bf16   (projection form B)
  scores S.T: [keys, queries] via paired K=64 matmuls (head pair at PE row
  offsets 0/64 with tile_position) into a 2-bank PSUM tile, one wide exp ACT
  AV: O.T accumulation with V_aug ones-column producing row sums; normalize
  via DVE fast reciprocal + GpSimd partition-broadcast.

Restructure vs the original baseline (398us -> ~388us):
  - input DMAs reordered (wk, kT pair-0, wq, q-chunks first; V and the
    rest interleaved behind) + minimal ramp so the first EXP fires ~23us
    in instead of ~45us
  - one flat 256-step pipeline at the EXP cadence; no per-block filler
    drains; AV jobs ride a tapering lag (8 steps early while the V
    projection streams in JIT, 5 in steady state, 2 at the end) so a
    v-starved AV can never head-of-line-block the scores/EXP stream
  - V input arrives as host-pretransposed per-seq-tile chunks [st, p, o*f]
    so each v chunk is one contiguous 2KB/partition DMA
  - lean tail: atc snapshot for dt0-2 of qb3, their out-proj partials
    precomputed (bf16) during the last steps, only 8 dt3 matmuls + adds
    gated on the final normalize

Dead ends measured on this hardware (do not retry): same-PSUM concurrent
row-split accumulation and 64-wide col-tiling both hang the PE
(NRT_EXEC_UNIT_UNRECOVERABLE; col-group 3 HW bug); matmul outputs cannot
span a PSUM bank (so 1024-wide moving with f32 psum is illegal); fp8
anywhere on the data path blows the 2e-2 error budget (~5% operand noise
passes straight through softmax); DVE has no exp/pow in hardware, so the
33.5M exps/core stay on ScalarE (~272us busy = the cadence floor); the
PSUM 8-bank budget (4 scores + 2 AV + 2 proj) pins the 512-query x
2-head step tiling and the [128,1024] EXP size.
"""

import time
from collections import defaultdict
from contextlib import ExitStack

import ml_dtypes
import numpy as np

import concourse.bass as bass
import concourse.mybir as mybir
import concourse.tile as tile
from concourse import bacc
from concourse.bass import ds, ts
from concourse.bass_utils import run_bass_kernel_spmd

F32 = mybir.dt.float32
BF16 = mybir.dt.bfloat16
EXP = mybir.ActivationFunctionType.Exp
MULT = mybir.AluOpType.mult
ADD = mybir.AluOpType.add
BF = ml_dtypes.bfloat16

B, S, D, H, DH = 4, 2048, 1024, 16, 64
HALF = D // 2          # 512 douts per core
DT = HALF // 128       # 4 dout tiles
DIN = D // 128         # 8 din tiles
QB = S // 512          # 4 query blocks
KT = S // 128          # 16 key tiles / seq tiles
STEPS = QB * DT * KT   # 256

TRACE = False
LAST_EXEC_NS = None
LAST_TRACE = None
_NC = None


def _build():
    nc = bacc.Bacc("TRN2", target_bir_lowering=False, debug=False,
                   num_devices=8, name="mha")

    qT_d = nc.dram_tensor("qT", [D, S], BF16, kind="ExternalInput")
    kT_d = nc.dram_tensor("kT", [D, S], BF16, kind="ExternalInput")
    vR_d = nc.dram_tensor("vR", [KT, 128, D], BF16, kind="ExternalInput")
    wq_d = nc.dram_tensor("wq", [D, HALF], BF16, kind="ExternalInput")
    wk_d = nc.dram_tensor("wk", [D, HALF], BF16, kind="ExternalInput")
    wv_d = nc.dram_tensor("wv", [D, HALF], BF16, kind="ExternalInput")
    wo_d = nc.dram_tensor("wo", [HALF, D], BF16, kind="ExternalInput")
    bq_d = nc.dram_tensor("bq", [1, HALF], BF16, kind="ExternalInput")
    bk_d = nc.dram_tensor("bk", [1, HALF], BF16, kind="ExternalInput")
    bv_d = nc.dram_tensor("bv", [1, HALF], BF16, kind="ExternalInput")
    out_d = nc.dram_tensor("out", [S, D], BF16, kind="ExternalOutput")

    stk = ExitStack()
    with tile.TileContext(nc) as tc:
        persist = stk.enter_context(tc.tile_pool(name="persist", bufs=1))
        kch = stk.enter_context(tc.tile_pool(name="kch", bufs=16))
        qch = stk.enter_context(tc.tile_pool(name="qch", bufs=16))
        vch = stk.enter_context(tc.tile_pool(name="vch", bufs=5))
        pTp = stk.enter_context(tc.tile_pool(name="pTp", bufs=11))
        otsb = stk.enter_context(tc.tile_pool(name="otsb", bufs=2))
        nrm = stk.enter_context(tc.tile_pool(name="nrm", bufs=1))
        outsb = stk.enter_context(tc.tile_pool(name="outsb", bufs=2))
        tailp = stk.enter_context(tc.tile_pool(name="tailp", bufs=8))
        ps_pair = stk.enter_context(tc.tile_pool(name="ps_pair", bufs=2, space="PSUM"))
        ps_ot = stk.enter_context(tc.tile_pool(name="ps_ot", bufs=2, space="PSUM"))
        ps_proj = stk.enter_context(tc.tile_pool(name="ps_proj", bufs=2, space="PSUM"))

        # --- persistent SBUF ---
        wq_sb = persist.tile([128, DIN, HALF], BF16)
        wk_sb = persist.tile([128, DIN, HALF], BF16)
        wv_sb = persist.tile([128, DIN, HALF], BF16)
        wo_sb = persist.tile([128, DT, D], BF16)
        bv_sb = persist.tile([1, HALF], BF16)
        bqP_bf = persist.tile([128, DT], BF16)
        bkP_bf = persist.tile([128, DT], BF16)
        bqP = persist.tile([128, DT], F32)
        bkP = persist.tile([128, DT], F32)
        bvb = persist.tile([128, HALF], BF16)
        qP = persist.tile([128, DT, S], BF16)
        kP = persist.tile([128, DT, S], BF16)
        v_aug = persist.tile([128, KT, 8 * 65], BF16)
        attnT = persist.tile([128, DT, S], BF16)
        atc = persist.tile([128, 3, 512], BF16)

        # ---- head DMAs, ordered so the first EXP fires ASAP ----
        nc.sync.dma_start(bkP_bf[:], bk_d[:].rearrange("a (o p) -> p (a o)", p=128))
        nc.sync.dma_start(bqP_bf[:], bq_d[:].rearrange("a (o p) -> p (a o)", p=128))
        nc.sync.dma_start(bv_sb[:], bv_d[:])
        nc.vector.tensor_copy(bkP[:], bkP_bf[:])
        nc.vector.tensor_copy(bqP[:], bqP_bf[:])
        nc.gpsimd.partition_broadcast(bvb[:], bv_sb[0:1, :])
        nc.sync.dma_start(wk_sb[:], wk_d[:].rearrange("(o p) n -> p o n", p=128))
        kchunks = {}
        for d in range(DIN):  # kb-pair 0: keys 0..1023
            t = kch.tile([128, 1024], BF16, tag="kch")
            nc.sync.dma_start(
                t[:], kT_d[:].rearrange("(o p) f -> o p f", p=128)[d][:, 0:1024])
            kchunks[(d, 0)] = t
        nc.sync.dma_start(wq_sb[:], wq_d[:].rearrange("(o p) n -> p o n", p=128))
        qchunks = {}

        def load_qchunks(qb):
            for d in range(DIN):
                t = qch.tile([128, 512], BF16, tag="qch")
                nc.sync.dma_start(
                    t[:], qT_d[:].rearrange("(o p) f -> o p f", p=128)[d][:, ts(qb, 512)])
                qchunks[(d, qb)] = t

        load_qchunks(0)
        nc.sync.dma_start(wv_sb[:], wv_d[:].rearrange("(o p) n -> p o n", p=128))
        nc.vector.memset(v_aug[:], 1.0)
        vchunks = {}

        def load_vchunk(st):
            t = vch.tile([128, DIN, 128], BF16, tag="vch")
            nc.sync.dma_start(
                t[:], vR_d[st].rearrange("p (o f) -> p o f", o=DIN))
            vchunks[st] = t

        def load_kpair1(ds_, de):
            for d in range(ds_, de):  # kb-pair 1: keys 1024..2047
                t = kch.tile([128, 1024], BF16, tag="kch")
                nc.sync.dma_start(
                    t[:], kT_d[:].rearrange("(o p) f -> o p f", p=128)[d][:, 1024:2048])
                kchunks[(d, 1)] = t

        for st in range(3):
            load_vchunk(st)
        load_kpair1(0, 4)
        for st in range(3, 5):
            load_vchunk(st)
        load_kpair1(4, 8)
        load_qchunks(1)

        # ---- projection emitters (item lists for the drip-feed) ----
        def kproj_items(dt, kb):
            """K projection, one dout tile x 512 keys + bias writeback."""
            state = {}

            def mk(d0):
                def mm():
                    if d0 == 0:
                        state["ps"] = ps_proj.tile([128, 512], F32, tag="proj", name="proj_ps")
                    for d in (d0, d0 + 1):
                        nc.tensor.matmul(state["ps"][:], wk_sb[:, d, ts(dt, 128)],
                                         kchunks[(d, kb // 2)][:, ts(kb % 2, 512)],
                                         start=(d == 0), stop=(d == DIN - 1))
                return mm

            def wb():
                nc.vector.tensor_scalar(kP[:, dt, ts(kb, 512)], state["ps"][:],
                                        bkP[:, dt:dt + 1], None, op0=ADD)

            return [mk(0), mk(2), mk(4), mk(6), wb]

        def qproj_items(dt, qb):
            """Q projection, one dout tile x 512 queries + bias writeback."""
            state = {}

            def mk(d0):
                def mm():
                    if d0 == 0:
                        state["ps"] = ps_proj.tile([128, 512], F32, tag="proj", name="proj_ps")
                    for d in (d0, d0 + 1):
                        nc.tensor.matmul(state["ps"][:], wq_sb[:, d, ts(dt, 128)],
                                         qchunks[(d, qb)][:],
                                         start=(d == 0), stop=(d == DIN - 1))
                return mm

            def wb():
                nc.vector.tensor_scalar(qP[:, dt, ts(qb, 512)], state["ps"][:],
                                        bqP[:, dt:dt + 1], None, op0=ADD)

            return [mk(0), mk(2), mk(4), mk(6), wb]

        def vproj_items(st):
            """V projection for seq tile st (all 8 local heads): 8 matmuls
            + biased writeback into v_aug's per-head 64-col slots."""
            state = {}

            def mk(d0):
                def mm():
                    if d0 == 0:
                        state["ps"] = ps_proj.tile([128, 512], F32, tag="proj", name="proj_ps")
                    for d in (d0, d0 + 1):
                        nc.tensor.matmul(state["ps"][:], vchunks[st][:, d, :],
                                         wv_sb[:, d, :],
                                         start=(d == 0), stop=(d == DIN - 1))
                return mm

            def wb():
                nc.vector.tensor_tensor(
                    v_aug[:, st].rearrange("p (h c) -> p h c", h=8)[:, :, 0:64],
                    state["ps"][:].rearrange("p (h c) -> p h c", h=8),
                    bvb[:].rearrange("p (h c) -> p h c", h=8), ADD)

            return [mk(0), mk(2), mk(4), mk(6), wb]

        def outproj_items(qb):
            """Out-projection for query block qb: per (seq tile, dout half),
            4 matmuls + writeback/DMA."""
            items = []
            for j in range(4):
                st = qb * 4 + j
                for half in range(2):
                    state = {}

                    def mk_a(st=st, half=half, state=state):
                        def mm():
                            state["ps"] = ps_proj.tile([128, 512], F32, tag="proj", name="proj_ps")
                            for dt in (0, 1):
                                nc.tensor.matmul(state["ps"][:],
                                                 attnT[:, dt, ts(st, 128)],
                                                 wo_sb[:, dt, ts(half, 512)],
                                                 start=(dt == 0), stop=False)
                        return mm

                    def mk_b(st=st, half=half, state=state):
                        def mm():
                            for dt in (2, 3):
                                nc.tensor.matmul(state["ps"][:],
                                                 attnT[:, dt, ts(st, 128)],
                                                 wo_sb[:, dt, ts(half, 512)],
                                                 start=False, stop=(dt == 3))
                        return mm

                    def wb(st=st, half=half, state=state):
                        osb = outsb.tile([128, 512], BF16, tag="osb")
                        nc.vector.tensor_copy(osb[:], state["ps"][:])
                        nc.sync.dma_start(
                            out_d[ds(st * 128, 128), ts(half, 512)], osb[:])

                    items += [mk_a(), mk_b(), wb]
            return items

        # ---- ramp: the minimum needed for scores step 0 ----
        for it in kproj_items(0, 0) + kproj_items(0, 1):
            it()
        for it in qproj_items(0, 0):
            it()

        # ---- filler schedule: step -> list of closures ----
        sched = defaultdict(list)

        def add(s0, items, per_step):
            s = s0
            n = 0
            for it in items:
                sched[s].append(it)
                n += 1
                if n >= per_step:
                    s += 1
                    n = 0

        # kP: dt needed from block hp=dt (step 16*dt); kbp1 of dt0 by step 8
        add(4, kproj_items(0, 2) + kproj_items(0, 3), 2)
        add(9, kproj_items(1, 0) + kproj_items(1, 1)
            + kproj_items(1, 2) + kproj_items(1, 3), 3)
        add(16, kproj_items(2, 0) + kproj_items(2, 1)
            + kproj_items(2, 2) + kproj_items(2, 3), 2)
        add(32, kproj_items(3, 0) + kproj_items(3, 1)
            + kproj_items(3, 2) + kproj_items(3, 3), 2)
        # V: group st feeds AV job (hp0, kt=st) which rides ~step st+6
        for st in range(16):
            add(st, vproj_items(st), 5)
        sched[2].append(lambda: [load_vchunk(st) for st in range(5, 9)])
        sched[6].append(lambda: [load_vchunk(st) for st in range(9, 13)])
        sched[10].append(lambda: [load_vchunk(st) for st in range(13, 16)])
        # Q: (dt, qb0) needed by block hp=dt; qb1.. later
        add(10, qproj_items(1, 0), 3)
        add(24, qproj_items(2, 0), 3)
        add(40, qproj_items(3, 0), 3)
        sched[20].append(lambda: nc.sync.dma_start(
            wo_sb[:], wo_d[:].rearrange("(o p) n -> p o n", p=128)))
        add(44, qproj_items(0, 1), 3)
        add(52, qproj_items(1, 1), 3)
        add(72, qproj_items(2, 1), 3)
        add(88, qproj_items(3, 1), 3)
        sched[64].append(lambda: load_qchunks(2))
        sched[140].append(lambda: load_qchunks(3))
        add(100, qproj_items(0, 2), 3)
        add(108, qproj_items(1, 2), 3)
        add(116, qproj_items(2, 2), 3)
        add(124, qproj_items(3, 2), 3)
        add(156, qproj_items(0, 3), 3)
        add(164, qproj_items(1, 3), 3)
        add(172, qproj_items(2, 3), 3)
        add(180, qproj_items(3, 3), 3)
        # out-projection of completed query blocks
        add(66, outproj_items(0), 1)
        add(130, outproj_items(1), 1)
        add(194, outproj_items(2), 1)

        # ---- AV job machinery (tapering lag behind the scores stream) ----
        pp_store = {}
        block_ot = {}
        av_state = {"idx": 0}

        def emit_normalize(b):
            qb_, hp_ = divmod(b, DT)
            otA, otB = block_ot.pop(b)
            sm = nrm.tile([1, 1024], F32, tag="sums")
            nc.vector.tensor_copy(sm[0:1, 0:512], otA[64:65, :])
            nc.vector.tensor_copy(sm[0:1, 512:1024], otB[64:65, :])
            r = nrm.tile([1, 1024], F32, tag="recip")
            nc.vector.reciprocal_approx_fast(r[0:1, :], sm[0:1, :])
            oa = otsb.tile([128, 512], F32, tag="ot_sb")
            ob = otsb.tile([128, 512], F32, tag="ot_sb")
            nc.vector.tensor_copy(oa[0:64, :], otA[0:64, :])
            nc.vector.tensor_copy(ob[0:64, :], otB[0:64, :])
            rb = nrm.tile([64, 1024], F32, tag="rb")
            nc.gpsimd.partition_broadcast(rb[:], r[0:1, :])
            nc.vector.tensor_tensor(attnT[0:64, hp_, ts(qb_, 512)],
                                    oa[0:64, :], rb[:, 0:512], MULT)
            nc.vector.tensor_tensor(attnT[64:128, hp_, ts(qb_, 512)],
                                    ob[0:64, :], rb[:, 512:1024], MULT)

        def emit_av():
            b, kt = divmod(av_state["idx"], KT)
            qb_, hp_ = divmod(b, DT)
            if kt == 0:
                block_ot[b] = (ps_ot.tile([128, 512], F32, tag="ot", name="otA"),
                               ps_ot.tile([128, 512], F32, tag="ot", name="otB"))
            otA, otB = block_ot[b]
            p = pp_store.pop((b, kt))
            nc.tensor.matmul(otA[0:65, :], v_aug[:, kt, ds(2 * hp_ * 65, 65)],
                             p[:, 0:512], start=(kt == 0), stop=(kt == KT - 1))
            nc.tensor.matmul(otB[0:65, :], v_aug[:, kt, ds((2 * hp_ + 1) * 65, 65)],
                             p[:, 512:1024], start=(kt == 0), stop=(kt == KT - 1))
            av_state["idx"] += 1
            if kt == KT - 1 and b < QB * DT - 1:
                emit_normalize(b)

        def av_target(s):
            if s < 30:
                lag = 8
            elif s < 40:
                lag = 6
            elif s < 248:
                lag = 5
            else:
                lag = 2
            return s - lag

        # ---- tail partials: dt0-2 of qb3's out-proj, dripped pre-tail ----
        tail_parts = {}

        def tail_partial_item(j):
            def f():
                psa = ps_proj.tile([128, 512], F32, tag="proj", name="tail_pa")
                psb = ps_proj.tile([128, 512], F32, tag="proj", name="tail_pb")
                for dt in (0, 1, 2):
                    nc.tensor.matmul(psa[:], atc[:, dt, ds(j * 128, 128)],
                                     wo_sb[:, dt, 0:512],
                                     start=(dt == 0), stop=(dt == 2))
                    nc.tensor.matmul(psb[:], atc[:, dt, ds(j * 128, 128)],
                                     wo_sb[:, dt, 512:1024],
                                     start=(dt == 0), stop=(dt == 2))
                tpa = tailp.tile([128, 512], BF16, tag="tp", name="tpa")
                tpb = tailp.tile([128, 512], BF16, tag="tp", name="tpb")
                nc.vector.tensor_copy(tpa[:], psa[:])
                nc.vector.tensor_copy(tpb[:], psb[:])
                tail_parts[(j, 0)] = tpa
                tail_parts[(j, 1)] = tpb
            return f

        sched[243].append(
            lambda: nc.vector.tensor_copy(atc[:], attnT[:, 0:3, ts(QB - 1, 512)]))
        for j in range(4):
            sched[245 + 2 * j].append(tail_partial_item(j))

        # ---- the flat 256-step pipeline ----
        for s in range(STEPS):
            qb, r = divmod(s, DT * KT)
            hp, kt = divmod(r, KT)
            pair = ps_pair.tile([128, 1024], F32, tag="pair")
            nc.tensor.matmul(pair[:, 0:512],
                             kP[0:64, hp, ts(kt, 128)],
                             qP[0:64, hp, ts(qb, 512)],
                             start=True, stop=True, tile_position=(0, 0))
            nc.tensor.matmul(pair[:, 512:1024],
                             kP[64:128, hp, ts(kt, 128)],
                             qP[64:128, hp, ts(qb, 512)],
                             start=True, stop=True, tile_position=(64, 0))
            p = pTp.tile([128, 1024], BF16, tag="pT")
            nc.scalar.activation(p[:], pair[:], EXP, scale=0.125)
            pp_store[(qb * DT + hp, kt)] = p
            for f in sched.pop(s, []):
                f()
            while av_state["idx"] <= av_target(s):
                emit_av()

        # ---- tail: last AV jobs, striped final normalize, dt3-only
        # out-proj matmuls added onto the precomputed partials ----
        while av_state["idx"] < QB * DT * KT:
            emit_av()
        b_last = QB * DT - 1
        otA, otB = block_ot.pop(b_last)
        sm = nrm.tile([1, 1024], F32, tag="sums")
        nc.vector.tensor_copy(sm[0:1, 0:512], otA[64:65, :])
        nc.vector.tensor_copy(sm[0:1, 512:1024], otB[64:65, :])
        r = nrm.tile([1, 1024], F32, tag="recip")
        nc.vector.reciprocal_approx_fast(r[0:1, :], sm[0:1, :])
        rb = nrm.tile([64, 1024], F32, tag="rb")
        nc.gpsimd.partition_broadcast(rb[:], r[0:1, :])
        oa = otsb.tile([128, 512], F32, tag="ot_sb")
        ob = otsb.tile([128, 512], F32, tag="ot_sb")
        nc.vector.tensor_copy(oa[0:64, :], otA[0:64, :])
        nc.vector.tensor_copy(ob[0:64, :], otB[0:64, :])
        for j in range(4):
            st = (QB - 1) * 4 + j
            nc.vector.tensor_tensor(
                attnT[0:64, 3, ds((QB - 1) * 512 + j * 128, 128)],
                oa[0:64, ts(j, 128)], rb[:, ts(j, 128)], MULT)
            nc.vector.tensor_tensor(
                attnT[64:128, 3, ds((QB - 1) * 512 + j * 128, 128)],
                ob[0:64, ts(j, 128)], rb[:, ds(512 + j * 128, 128)], MULT)
            for half in range(2):
                tps = ps_proj.tile([128, 512], F32, tag="proj", name="tail_ps")
                nc.tensor.matmul(tps[:], attnT[:, 3, ts(st, 128)],
                                 wo_sb[:, 3, ts(half, 512)],
                                 start=True, stop=True)
                osb = outsb.tile([128, 512], BF16, tag="osb")
                nc.vector.tensor_tensor(osb[:], tail_parts[(j, half)][:],
                                        tps[:], ADD)
                nc.sync.dma_start(out_d[ds(st * 128, 128), ts(half, 512)], osb[:])

        stk.close()

    nc.finalize()
    return nc


def kernel(q, k, v, mask, w_q, b_q, w_k, b_k, w_v, b_v, w_o, b_o):
    global _NC, LAST_EXEC_NS, LAST_TRACE
    if _NC is None:
        _NC = _build()
    nc = _NC

    q = np.asarray(q, np.float32)
    k = np.asarray(k, np.float32)
    v = np.asarray(v, np.float32)
    w_q = np.asarray(w_q, np.float32)
    w_k = np.asarray(w_k, np.float32)
    w_v = np.asarray(w_v, np.float32)
    w_o = np.asarray(w_o, np.float32)
    b_q = np.asarray(b_q, np.float32)
    b_k = np.asarray(b_k, np.float32)
    b_v = np.asarray(b_v, np.float32)
    b_o = np.asarray(b_o, np.float32)

    in_maps = []
    for c in range(8):
        b, hf = divmod(c, 2)
        sl = slice(hf * HALF, (hf + 1) * HALF)
        # vR[st, p, o, f] = v[b][st*128+f, o*128+p]: one contiguous
        # 2KB/partition DMA per seq tile
        vR = np.ascontiguousarray(
            v[b].reshape(KT, 128, DIN, 128).transpose(0, 3, 2, 1)).astype(BF)
        in_maps.append({
            "qT": q[b].T.astype(BF),
            "kT": k[b].T.astype(BF),
            "vR": vR,
            "wq": w_q[sl, :].T.astype(BF),
            "wk": w_k[sl, :].T.astype(BF),
            "wv": w_v[sl, :].T.astype(BF),
            "wo": w_o[:, sl].T.astype(BF),
            "bq": b_q[sl].reshape(1, HALF).astype(BF),
            "bk": b_k[sl].reshape(1, HALF).astype(BF),
            "bv": b_v[sl].reshape(1, HALF).astype(BF),
        })

    kwargs = {}
    if TRACE:
        kwargs = dict(trace=True, trace_cores=[0])
    try:
        res = run_bass_kernel_spmd(nc, in_maps, core_ids=list(range(8)), **kwargs)
    except Exception:
        # transient device wedge usually clears on retry
        time.sleep(2.0)
        res = run_bass_kernel_spmd(nc, in_maps, core_ids=list(range(8)), **kwargs)
    if TRACE:
        LAST_EXEC_NS = res.exec_time_ns
        LAST_TRACE = res.instructions_and_trace[1] if res.instructions_and_trace else None

    out = np.empty((B, S, D), np.float32)
    for b in range(B):
        out[b] = (res.results[2 * b]["out"].astype(np.float32)
                  + res.results[2 * b + 1]["out"].astype(np.float32)
                  + b_o[None, :])
    return out


# revision 32
# speedup vs baseline: 1.0215x; 1.0029x over previous
"""Multi-head attention (B=4, S=2048, D=1024, H=16) on 8 Trainium2 cores.

Sharding: data parallel on batch (4) x tensor parallel on heads (2 halves of
8 heads). Core c handles batch c//2 and head-half c%2: column-parallel
w_q/w_k/w_v (512 out dims), local attention over its 8 heads, row-parallel
w_o (its 512 hd columns) producing a full [2048, 1024] partial that the host
sums across the two halves (plus b_o).

On-device layout is feature-on-partitions throughout ("transposed"):
  qP/kP: [dout 512 -> 4 ptiles, seq 2048] bf16   (projection form B)
  scores S.T: [keys, queries] via paired K=64 matmuls (head pair at PE row
  offsets 0/64 with tile_position) into a 2-bank PSUM tile, one wide exp ACT
  AV: O.T accumulation with V_aug ones-column producing row sums; normalize
  via DVE fast reciprocal + GpSimd partition-broadcast.

Restructure vs the original baseline (398us -> ~388us):
  - input DMAs reordered (wk, kT pair-0, wq, q-chunks first; V and the
    rest interleaved behind) + minimal ramp so the first EXP fires ~23us
    in instead of ~45us
  - one flat 256-step pipeline at the EXP cadence; no per-block filler
    drains; AV jobs ride a tapering lag (8 steps early while the V
    projection streams in JIT, 5 in steady state, 2 at the end) so a
    v-starved AV can never head-of-line-block the scores/EXP stream
  - V input arrives as host-pretransposed per-seq-tile chunks [st, p, o*f]
    so each v chunk is one contiguous 2KB/partition DMA
  - lean tail: atc snapshot for dt0-2 of qb3, their out-proj partials
    precomputed (bf16) during the last steps, only 8 dt3 matmuls + adds
    gated on the final normalize

Dead ends measured on this hardware (do not retry): same-PSUM concurrent
row-split accumulation and 64-wide col-tiling both hang the PE
(NRT_EXEC_UNIT_UNRECOVERABLE; col-group 3 HW bug); matmul outputs cannot
span a PSUM bank (so 1024-wide moving with f32 psum is illegal); fp8
anywhere on the data path blows the 2e-2 error budget (~5% operand noise
passes straight through softmax); DVE has no exp/pow in hardware, so the
33.5M exps/core stay on ScalarE (~272us busy = the cadence floor); the
PSUM 8-bank budget (4 scores + 2 AV + 2 proj) pins the 512-query x
2-head step tiling and the [128,1024] EXP size.
"""

import time
from collections import defaultdict
from contextlib import ExitStack

import ml_dtypes
import numpy as np

import concourse.bass as bass
import concourse.mybir as mybir
import concourse.tile as tile
from concourse import bacc
from concourse.bass import ds, ts
from concourse.bass_utils import run_bass_kernel_spmd

F32 = mybir.dt.float32
BF16 = mybir.dt.bfloat16
EXP = mybir.ActivationFunctionType.Exp
MULT = mybir.AluOpType.mult
ADD = mybir.AluOpType.add
BF = ml_dtypes.bfloat16

B, S, D, H, DH = 4, 2048, 1024, 16, 64
HALF = D // 2          # 512 douts per core
DT = HALF // 128       # 4 dout tiles
DIN = D // 128         # 8 din tiles
QB = S // 512          # 4 query blocks
KT = S // 128          # 16 key tiles / seq tiles
STEPS = QB * DT * KT   # 256

TRACE = False
LAST_EXEC_NS = None
LAST_TRACE = None
_NC = None


def _build():
    nc = bacc.Bacc("TRN2", target_bir_lowering=False, debug=False,
                   num_devices=8, name="mha")

    qT_d = nc.dram_tensor("qT", [D, S], BF16, kind="ExternalInput")
    kT_d = nc.dram_tensor("kT", [D, S], BF16, kind="ExternalInput")
    vR_d = nc.dram_tensor("vR", [KT, 128, D], BF16, kind="ExternalInput")
    wq_d = nc.dram_tensor("wq", [D, HALF], BF16, kind="ExternalInput")
    wk_d = nc.dram_tensor("wk", [D, HALF], BF16, kind="ExternalInput")
    wv_d = nc.dram_tensor("wv", [D, HALF], BF16, kind="ExternalInput")
    wo_d = nc.dram_tensor("wo", [HALF, D], BF16, kind="ExternalInput")
    bq_d = nc.dram_tensor("bq", [1, HALF], BF16, kind="ExternalInput")
    bk_d = nc.dram_tensor("bk", [1, HALF], BF16, kind="ExternalInput")
    bv_d = nc.dram_tensor("bv", [1, HALF], BF16, kind="ExternalInput")
    out_d = nc.dram_tensor("out", [S, D], BF16, kind="ExternalOutput")

    stk = ExitStack()
    with tile.TileContext(nc) as tc:
        persist = stk.enter_context(tc.tile_pool(name="persist", bufs=1))
        kch = stk.enter_context(tc.tile_pool(name="kch", bufs=16))
        qch = stk.enter_context(tc.tile_pool(name="qch", bufs=16))
        vch = stk.enter_context(tc.tile_pool(name="vch", bufs=5))
        pTp = stk.enter_context(tc.tile_pool(name="pTp", bufs=11))
        otsb = stk.enter_context(tc.tile_pool(name="otsb", bufs=2))
        nrm = stk.enter_context(tc.tile_pool(name="nrm", bufs=1))
        outsb = stk.enter_context(tc.tile_pool(name="outsb", bufs=2))
        tailp = stk.enter_context(tc.tile_pool(name="tailp", bufs=8))
        ps_pair = stk.enter_context(tc.tile_pool(name="ps_pair", bufs=2, space="PSUM"))
        ps_ot = stk.enter_context(tc.tile_pool(name="ps_ot", bufs=2, space="PSUM"))
        ps_proj = stk.enter_context(tc.tile_pool(name="ps_proj", bufs=2, space="PSUM"))

        # --- persistent SBUF ---
        wq_sb = persist.tile([128, DIN, HALF], BF16)
        wk_sb = persist.tile([128, DIN, HALF], BF16)
        wv_sb = persist.tile([128, DIN, HALF], BF16)
        wo_sb = persist.tile([128, DT, D], BF16)
        bv_sb = persist.tile([1, HALF], BF16)
        bqP_bf = persist.tile([128, DT], BF16)
        bkP_bf = persist.tile([128, DT], BF16)
        bqP = persist.tile([128, DT], F32)
        bkP = persist.tile([128, DT], F32)
        bvb = persist.tile([128, HALF], BF16)
        qP = persist.tile([128, DT, S], BF16)
        kP = persist.tile([128, DT, S], BF16)
        v_aug = persist.tile([128, KT, 8 * 65], BF16)
        attnT = persist.tile([128, DT, S], BF16)
        atc = persist.tile([128, 3, 512], BF16)

        # ---- head DMAs, ordered so the first EXP fires ASAP ----
        nc.sync.dma_start(bkP_bf[:], bk_d[:].rearrange("a (o p) -> p (a o)", p=128))
        nc.sync.dma_start(bqP_bf[:], bq_d[:].rearrange("a (o p) -> p (a o)", p=128))
        nc.sync.dma_start(bv_sb[:], bv_d[:])
        nc.vector.tensor_copy(bkP[:], bkP_bf[:])
        nc.vector.tensor_copy(bqP[:], bqP_bf[:])
        nc.gpsimd.partition_broadcast(bvb[:], bv_sb[0:1, :])
        nc.sync.dma_start(wq_sb[:], wq_d[:].rearrange("(o p) n -> p o n", p=128))
        qchunks = {}

        def load_qchunks(qb):
            for d in range(DIN):
                t = qch.tile([128, 512], BF16, tag="qch")
                nc.sync.dma_start(
                    t[:], qT_d[:].rearrange("(o p) f -> o p f", p=128)[d][:, ts(qb, 512)])
                qchunks[(d, qb)] = t

        load_qchunks(0)
        nc.sync.dma_start(wk_sb[:], wk_d[:].rearrange("(o p) n -> p o n", p=128))
        kchunks = {}
        for d in range(DIN):  # kb-pair 0: keys 0..1023
            t = kch.tile([128, 1024], BF16, tag="kch")
            nc.sync.dma_start(
                t[:], kT_d[:].rearrange("(o p) f -> o p f", p=128)[d][:, 0:1024])
            kchunks[(d, 0)] = t
        nc.sync.dma_start(wv_sb[:], wv_d[:].rearrange("(o p) n -> p o n", p=128))
        nc.vector.memset(v_aug[:], 1.0)
        vchunks = {}

        def load_vchunk(st):
            t = vch.tile([128, DIN, 128], BF16, tag="vch")
            nc.sync.dma_start(
                t[:], vR_d[st].rearrange("p (o f) -> p o f", o=DIN))
            vchunks[st] = t

        def load_kpair1(ds_, de):
            for d in range(ds_, de):  # kb-pair 1: keys 1024..2047
                t = kch.tile([128, 1024], BF16, tag="kch")
                nc.sync.dma_start(
                    t[:], kT_d[:].rearrange("(o p) f -> o p f", p=128)[d][:, 1024:2048])
                kchunks[(d, 1)] = t

        for st in range(3):
            load_vchunk(st)
        load_kpair1(0, 4)
        for st in range(3, 5):
            load_vchunk(st)
        load_kpair1(4, 8)
        load_qchunks(1)

        # ---- projection emitters (item lists for the drip-feed) ----
        def kproj_items(dt, kb):
            """K projection, one dout tile x 512 keys + bias writeback."""
            state = {}

            def mk(d0):
                def mm():
                    if d0 == 0:
                        state["ps"] = ps_proj.tile([128, 512], F32, tag="proj", name="proj_ps")
                    for d in (d0, d0 + 1):
                        nc.tensor.matmul(state["ps"][:], wk_sb[:, d, ts(dt, 128)],
                                         kchunks[(d, kb // 2)][:, ts(kb % 2, 512)],
                                         start=(d == 0), stop=(d == DIN - 1))
                return mm

            def wb():
                nc.vector.tensor_scalar(kP[:, dt, ts(kb, 512)], state["ps"][:],
                                        bkP[:, dt:dt + 1], None, op0=ADD)

            return [mk(0), mk(2), mk(4), mk(6), wb]

        def qproj_items(dt, qb):
            """Q projection, one dout tile x 512 queries + bias writeback."""
            state = {}

            def mk(d0):
                def mm():
                    if d0 == 0:
                        state["ps"] = ps_proj.tile([128, 512], F32, tag="proj", name="proj_ps")
                    for d in (d0, d0 + 1):
                        nc.tensor.matmul(state["ps"][:], wq_sb[:, d, ts(dt, 128)],
                                         qchunks[(d, qb)][:],
                                         start=(d == 0), stop=(d == DIN - 1))
                return mm

            def wb():
                nc.vector.tensor_scalar(qP[:, dt, ts(qb, 512)], state["ps"][:],
                                        bqP[:, dt:dt + 1], None, op0=ADD)

            return [mk(0), mk(2), mk(4), mk(6), wb]

        def vproj_items(st):
            """V projection for seq tile st (all 8 local heads): 8 matmuls
            + biased writeback into v_aug's per-head 64-col slots."""
            state = {}

            def mk(d0):
                def mm():
                    if d0 == 0:
                        state["ps"] = ps_proj.tile([128, 512], F32, tag="proj", name="proj_ps")
                    for d in (d0, d0 + 1):
                        nc.tensor.matmul(state["ps"][:], vchunks[st][:, d, :],
                                         wv_sb[:, d, :],
                                         start=(d == 0), stop=(d == DIN - 1))
                return mm

            def wb():
                nc.vector.tensor_tensor(
                    v_aug[:, st].rearrange("p (h c) -> p h c", h=8)[:, :, 0:64],
                    state["ps"][:].rearrange("p (h c) -> p h c", h=8),
                    bvb[:].rearrange("p (h c) -> p h c", h=8), ADD)

            return [mk(0), mk(2), mk(4), mk(6), wb]

        def outproj_items(qb):
            """Out-projection for query block qb: per (seq tile, dout half),
            4 matmuls + writeback/DMA."""
            items = []
            for j in range(4):
                st = qb * 4 + j
                for half in range(2):
                    state = {}

                    def mk_a(st=st, half=half, state=state):
                        def mm():
                            state["ps"] = ps_proj.tile([128, 512], F32, tag="proj", name="proj_ps")
                            for dt in (0, 1):
                                nc.tensor.matmul(state["ps"][:],
                                                 attnT[:, dt, ts(st, 128)],
                                                 wo_sb[:, dt, ts(half, 512)],
                                                 start=(dt == 0), stop=False)
                        return mm

                    def mk_b(st=st, half=half, state=state):
                        def mm():
                            for dt in (2, 3):
                                nc.tensor.matmul(state["ps"][:],
                                                 attnT[:, dt, ts(st, 128)],
                                                 wo_sb[:, dt, ts(half, 512)],
                                                 start=False, stop=(dt == 3))
                        return mm

                    def wb(st=st, half=half, state=state):
                        osb = outsb.tile([128, 512], BF16, tag="osb")
                        nc.vector.tensor_copy(osb[:], state["ps"][:])
                        nc.sync.dma_start(
                            out_d[ds(st * 128, 128), ts(half, 512)], osb[:])

                    items += [mk_a(), mk_b(), wb]
            return items

        # ---- ramp: the minimum needed for scores step 0 ----
        for it in qproj_items(0, 0):
            it()
        for it in kproj_items(0, 0) + kproj_items(0, 1):
            it()

        # ---- filler schedule: step -> list of closures ----
        sched = defaultdict(list)

        def add(s0, items, per_step):
            s = s0
            n = 0
            for it in items:
                sched[s].append(it)
                n += 1
                if n >= per_step:
                    s += 1
                    n = 0

        # kP: dt needed from block hp=dt (step 16*dt); kbp1 of dt0 by step 8
        add(4, kproj_items(0, 2) + kproj_items(0, 3), 2)
        add(9, kproj_items(1, 0) + kproj_items(1, 1)
            + kproj_items(1, 2) + kproj_items(1, 3), 3)
        add(16, kproj_items(2, 0) + kproj_items(2, 1)
            + kproj_items(2, 2) + kproj_items(2, 3), 2)
        add(32, kproj_items(3, 0) + kproj_items(3, 1)
            + kproj_items(3, 2) + kproj_items(3, 3), 2)
        # V: group st feeds AV job (hp0, kt=st) which rides ~step st+6
        for st in range(16):
            add(st, vproj_items(st), 5)
        sched[2].append(lambda: [load_vchunk(st) for st in range(5, 9)])
        sched[6].append(lambda: [load_vchunk(st) for st in range(9, 13)])
        sched[10].append(lambda: [load_vchunk(st) for st in range(13, 16)])
        # Q: (dt, qb0) needed by block hp=dt; qb1.. later
        add(10, qproj_items(1, 0), 3)
        add(24, qproj_items(2, 0), 3)
        add(40, qproj_items(3, 0), 3)
        sched[20].append(lambda: nc.sync.dma_start(
            wo_sb[:], wo_d[:].rearrange("(o p) n -> p o n", p=128)))
        add(44, qproj_items(0, 1), 3)
        add(52, qproj_items(1, 1), 3)
        add(72, qproj_items(2, 1), 3)
        add(88, qproj_items(3, 1), 3)
        sched[64].append(lambda: load_qchunks(2))
        sched[140].append(lambda: load_qchunks(3))
        add(100, qproj_items(0, 2), 3)
        add(108, qproj_items(1, 2), 3)
        add(116, qproj_items(2, 2), 3)
        add(124, qproj_items(3, 2), 3)
        add(156, qproj_items(0, 3), 3)
        add(164, qproj_items(1, 3), 3)
        add(172, qproj_items(2, 3), 3)
        add(180, qproj_items(3, 3), 3)
        # out-projection of completed query blocks
        add(66, outproj_items(0), 1)
        add(130, outproj_items(1), 1)
        add(194, outproj_items(2), 1)

        # ---- AV job machinery (tapering lag behind the scores stream) ----
        pp_store = {}
        block_ot = {}
        av_state = {"idx": 0}

        def emit_normalize(b):
            qb_, hp_ = divmod(b, DT)
            otA, otB = block_ot.pop(b)
            sm = nrm.tile([1, 1024], F32, tag="sums")
            nc.vector.tensor_copy(sm[0:1, 0:512], otA[64:65, :])
            nc.vector.tensor_copy(sm[0:1, 512:1024], otB[64:65, :])
            r = nrm.tile([1, 1024], F32, tag="recip")
            nc.vector.reciprocal_approx_fast(r[0:1, :], sm[0:1, :])
            oa = otsb.tile([128, 512], F32, tag="ot_sb")
            ob = otsb.tile([128, 512], F32, tag="ot_sb")
            nc.vector.tensor_copy(oa[0:64, :], otA[0:64, :])
            nc.vector.tensor_copy(ob[0:64, :], otB[0:64, :])
            rb = nrm.tile([64, 1024], F32, tag="rb")
            nc.gpsimd.partition_broadcast(rb[:], r[0:1, :])
            nc.vector.tensor_tensor(attnT[0:64, hp_, ts(qb_, 512)],
                                    oa[0:64, :], rb[:, 0:512], MULT)
            nc.vector.tensor_tensor(attnT[64:128, hp_, ts(qb_, 512)],
                                    ob[0:64, :], rb[:, 512:1024], MULT)

        def emit_av():
            b, kt = divmod(av_state["idx"], KT)
            qb_, hp_ = divmod(b, DT)
            if kt == 0:
                block_ot[b] = (ps_ot.tile([128, 512], F32, tag="ot", name="otA"),
                               ps_ot.tile([128, 512], F32, tag="ot", name="otB"))
            otA, otB = block_ot[b]
            p = pp_store.pop((b, kt))
            nc.tensor.matmul(otA[0:65, :], v_aug[:, kt, ds(2 * hp_ * 65, 65)],
                             p[:, 0:512], start=(kt == 0), stop=(kt == KT - 1))
            nc.tensor.matmul(otB[0:65, :], v_aug[:, kt, ds((2 * hp_ + 1) * 65, 65)],
                             p[:, 512:1024], start=(kt == 0), stop=(kt == KT - 1))
            av_state["idx"] += 1
            if kt == KT - 1 and b < QB * DT - 1:
                emit_normalize(b)

        def av_target(s):
            if s < 30:
                lag = 8
            elif s < 40:
                lag = 6
            elif s < 248:
                lag = 5
            else:
                lag = 2
            return s - lag

        # ---- tail partials: dt0-2 of qb3's out-proj, dripped pre-tail ----
        tail_parts = {}

        def tail_partial_item(j):
            def f():
                psa = ps_proj.tile([128, 512], F32, tag="proj", name="tail_pa")
                psb = ps_proj.tile([128, 512], F32, tag="proj", name="tail_pb")
                for dt in (0, 1, 2):
                    nc.tensor.matmul(psa[:], atc[:, dt, ds(j * 128, 128)],
                                     wo_sb[:, dt, 0:512],
                                     start=(dt == 0), stop=(dt == 2))
                    nc.tensor.matmul(psb[:], atc[:, dt, ds(j * 128, 128)],
                                     wo_sb[:, dt, 512:1024],
                                     start=(dt == 0), stop=(dt == 2))
                tpa = tailp.tile([128, 512], BF16, tag="tp", name="tpa")
                tpb = tailp.tile([128, 512], BF16, tag="tp", name="tpb")
                nc.vector.tensor_copy(tpa[:], psa[:])
                nc.vector.tensor_copy(tpb[:], psb[:])
                tail_parts[(j, 0)] = tpa
                tail_parts[(j, 1)] = tpb
            return f

        sched[243].append(
            lambda: nc.vector.tensor_copy(atc[:], attnT[:, 0:3, ts(QB - 1, 512)]))
        for j in range(4):
            sched[245 + 2 * j].append(tail_partial_item(j))

        # ---- the flat 256-step pipeline ----
        for s in range(STEPS):
            qb, r = divmod(s, DT * KT)
            hp, kt = divmod(r, KT)
            pair = ps_pair.tile([128, 1024], F32, tag="pair")
            nc.tensor.matmul(pair[:, 0:512],
                             kP[0:64, hp, ts(kt, 128)],
                             qP[0:64, hp, ts(qb, 512)],
                             start=True, stop=True, tile_position=(0, 0))
            nc.tensor.matmul(pair[:, 512:1024],
                             kP[64:128, hp, ts(kt, 128)],
                             qP[64:128, hp, ts(qb, 512)],
                             start=True, stop=True, tile_position=(64, 0))
            p = pTp.tile([128, 1024], BF16, tag="pT")
            nc.scalar.activation(p[:], pair[:], EXP, scale=0.125)
            pp_store[(qb * DT + hp, kt)] = p
            for f in sched.pop(s, []):
                f()
            while av_state["idx"] <= av_target(s):
                emit_av()

        # ---- tail: last AV jobs, striped final normalize, dt3-only
        # out-proj matmuls added onto the precomputed partials ----
        while av_state["idx"] < QB * DT * KT:
            emit_av()
        b_last = QB * DT - 1
        otA, otB = block_ot.pop(b_last)
        sm = nrm.tile([1, 1024], F32, tag="sums")
        nc.vector.tensor_copy(sm[0:1, 0:512], otA[64:65, :])
        nc.vector.tensor_copy(sm[0:1, 512:1024], otB[64:65, :])
        r = nrm.tile([1, 1024], F32, tag="recip")
        nc.vector.reciprocal_approx_fast(r[0:1, :], sm[0:1, :])
        rb = nrm.tile([64, 1024], F32, tag="rb")
        nc.gpsimd.partition_broadcast(rb[:], r[0:1, :])
        oa = otsb.tile([128, 512], F32, tag="ot_sb")
        ob = otsb.tile([128, 512], F32, tag="ot_sb")
        nc.vector.tensor_copy(oa[0:64, :], otA[0:64, :])
        nc.vector.tensor_copy(ob[0:64, :], otB[0:64, :])
        for j in range(4):
            st = (QB - 1) * 4 + j
            nc.vector.tensor_tensor(
                attnT[0:64, 3, ds((QB - 1) * 512 + j * 128, 128)],
                oa[0:64, ts(j, 128)], rb[:, ts(j, 128)], MULT)
            nc.vector.tensor_tensor(
                attnT[64:128, 3, ds((QB - 1) * 512 + j * 128, 128)],
                ob[0:64, ts(j, 128)], rb[:, ds(512 + j * 128, 128)], MULT)
            for half in range(2):
                tps = ps_proj.tile([128, 512], F32, tag="proj", name="tail_ps")
                nc.tensor.matmul(tps[:], attnT[:, 3, ts(st, 128)],
                                 wo_sb[:, 3, ts(half, 512)],
                                 start=True, stop=True)
                osb = outsb.tile([128, 512], BF16, tag="osb")
                nc.vector.tensor_tensor(osb[:], tail_parts[(j, half)][:],
                                        tps[:], ADD)
                nc.sync.dma_start(out_d[ds(st * 128, 128), ts(half, 512)], osb[:])

        stk.close()

    nc.finalize()
    return nc


def kernel(q, k, v, mask, w_q, b_q, w_k, b_k, w_v, b_v, w_o, b_o):
    global _NC, LAST_EXEC_NS, LAST_TRACE
    if _NC is None:
        _NC = _build()
    nc = _NC

    q = np.asarray(q, np.float32)
    k = np.asarray(k, np.float32)
    v = np.asarray(v, np.float32)
    w_q = np.asarray(w_q, np.float32)
    w_k = np.asarray(w_k, np.float32)
    w_v = np.asarray(w_v, np.float32)
    w_o = np.asarray(w_o, np.float32)
    b_q = np.asarray(b_q, np.float32)
    b_k = np.asarray(b_k, np.float32)
    b_v = np.asarray(b_v, np.float32)
    b_o = np.asarray(b_o, np.float32)

    in_maps = []
    for c in range(8):
        b, hf = divmod(c, 2)
        sl = slice(hf * HALF, (hf + 1) * HALF)
        # vR[st, p, o, f] = v[b][st*128+f, o*128+p]: one contiguous
        # 2KB/partition DMA per seq tile
        vR = np.ascontiguousarray(
            v[b].reshape(KT, 128, DIN, 128).transpose(0, 3, 2, 1)).astype(BF)
        in_maps.append({
            "qT": q[b].T.astype(BF),
            "kT": k[b].T.astype(BF),
            "vR": vR,
            "wq": w_q[sl, :].T.astype(BF),
            "wk": w_k[sl, :].T.astype(BF),
            "wv": w_v[sl, :].T.astype(BF),
            "wo": w_o[:, sl].T.astype(BF),
            "bq": b_q[sl].reshape(1, HALF).astype(BF),
            "bk": b_k[sl].reshape(1, HALF).astype(BF),
            "bv": b_v[sl].reshape(1, HALF).astype(BF),
        })

    kwargs = {}
    if TRACE:
        kwargs = dict(trace=True, trace_cores=[0])
    try:
        res = run_bass_kernel_spmd(nc, in_maps, core_ids=list(range(8)), **kwargs)
    except Exception:
        # transient device wedge usually clears on retry
        time.sleep(2.0)
        res = run_bass_kernel_spmd(nc, in_maps, core_ids=list(range(8)), **kwargs)
    if TRACE:
        LAST_EXEC_NS = res.exec_time_ns
        LAST_TRACE = res.instructions_and_trace[1] if res.instructions_and_trace else None

    out = np.empty((B, S, D), np.float32)
    for b in range(B):
        out[b] = (res.results[2 * b]["out"].astype(np.float32)
                  + res.results[2 * b + 1]["out"].astype(np.float32)
                  + b_o[None, :])
    return out
